# revision 52
# baseline (speedup 1.0000x reference)
"""Trainium2 Bass kernel for nn_CorrelationMapLayer.

reference semantics:
    d1 = bilinear_down28(feature1)            # [B, C, 28, 28]
    d2 = bilinear_down28(feature2)            # [B, C, 28, 28]
    f2_sel[b,c,k] = d2[b, c, y_k, x_k]        # knn gather (y=knn[:,1], x=knn[:,0])
    corr = relu(einsum('bck,bchw->bkhw', f2_sel, d1))
    out  = corr / sum_{h,w} exp(corr) * 10

Kernel structure (final):
  * inputs are cast to f16 on the host; DMA is the roofline.
  * f2 is consumed only through the 4 bilinear taps at the K knn points
    (0.5% of the tensor). The host-side shard step slices those tap
    columns out of f2 (pure indexing -- the bilinear weights and all
    arithmetic stay on device), so each core loads ~13MB instead of ~26MB.
  * All 16 tap-combine stages (one mul + two pair adds per channel block)
    are hoisted ahead of the main loop so the PE matmul stream only ever
    waits on f1 quarter arrivals.
  * f1 feeds the correlation matmul in the original 56x56 space as raw
    f16 (no elementwise work, 4 quarter-DMAs per batch); the bilinear
    downsample is applied AFTER the matmul on corr56 [K=100, 3136]
    (K < C so this is ~5x cheaper than premultiplying f1): psum -> f16
    copy (ACT/DVE alternating), premultiply by the separable weight map,
    h-pair add, strided w-pair add (2 of 7 groups on gpsimd), then
    relu / exp+accumulate / reciprocal / scale-by-10.
  * Queues: input DMAs on the SP HWDGE queue, output + consts on the ACT
    HWDGE queue so neither stream head-of-line blocks the other.
  * Data parallel over batch: 4 batches per core x 8 cores.
"""

import os
import sys

import numpy as np

for _p in (
    "/root/.axon_site",
    "/root/.axon_site/_ro/trn_rl_repo",
    "/root/.axon_site/_ro/pypackages",
    "/opt/trn_rl_repo",
):
    if os.path.isdir(_p) and _p not in sys.path:
        sys.path.append(_p)

import concourse.bacc as bacc
import concourse.mybir as mybir
import concourse.tile as tile
from concourse import bass_utils

F32 = mybir.dt.float32
BF16 = mybir.dt.bfloat16
FP8 = mybir.dt.float8e4
F16 = mybir.dt.float16
F16_NP = mybir.dt.np(mybir.dt.float16)
FP8_NP = mybir.dt.np(mybir.dt.float8e4)
F1SCALE = 1.0
AF = mybir.ActivationFunctionType

B, C, H, W, K = 32, 512, 56, 56, 100
NCORES = 8
BL = B // NCORES  # batches per core
S = 28
HW = H * W  # 3136
HW28 = S * S  # 784
NCB = C // 128  # 4 channel blocks
NG = 7  # corr h-row groups (7 x 8 original rows)
F1ROWS = [0] + list(range(2, 54)) + [55]  # 54 shipped rows (drop 1, 54)
HS = len(F1ROWS)  # 54
# per corr group: (quarter idx, local row start, shipped row count)
GQ = [(0, 0, 7), (0, 7, 8), (1, 0, 8), (1, 8, 8), (2, 0, 8), (2, 8, 8), (3, 0, 7)]
GSTART = [0, 7, 15, 23, 31, 39, 47]  # shipped row offset of each group

BF16_NP = mybir.dt.np(BF16)


def _bilinear_matrix(in_size: int, out_size: int) -> np.ndarray:
    scale = np.float32((in_size - 1) / (out_size - 1)) if out_size > 1 else np.float32(0)
    coords = np.arange(out_size, dtype=np.float32) * scale
    lo = np.floor(coords).astype(np.int32)
    hi = np.minimum(lo + 1, in_size - 1)
    frac = coords - lo.astype(np.float32)
    M = np.zeros((out_size, in_size), np.float32)
    np.add.at(M, (np.arange(out_size), lo), np.float32(1.0) - frac)
    np.add.at(M, (np.arange(out_size), hi), frac)
    return M


def _tap_weights() -> np.ndarray:
    """wvec[w]: weight applied to input index w, whose (unique) consumer is
    output index w//2. Verifies the 2-tap stride-2 structure exactly."""
    M = _bilinear_matrix(H, S)  # [28, 56]
    wvec = np.zeros(H, np.float32)
    for w in range(H):
        wvec[w] = M[w // 2, w]
    M2 = np.zeros_like(M)
    for ow in range(S):
        M2[ow, 2 * ow] = wvec[2 * ow]
        M2[ow, 2 * ow + 1] = wvec[2 * ow + 1]
    assert np.abs(M - M2).max() <= 1e-6, "bilinear 2-tap structure violated"
    return wvec


_WVEC = _tap_weights()
# WF[p, h*56+w] = wvec[h]*wvec[w]  (full separable 2D weight map)
_WF_ROW = (np.repeat(_WVEC, W) * np.tile(_WVEC, H)).astype(np.float32)
_WF54 = _WF_ROW.reshape(H, W)[[0] + list(range(2, 54)) + [55], :].reshape(-1)
WF_NP = np.ascontiguousarray(
    np.broadcast_to(_WF54[None, :], (128, 54 * W)), dtype=F16_NP
)


def _tap_tables(knn_inds: np.ndarray):
    """Flat hw indices of the 4 bilinear taps per knn point (for the host
    slice) + the matching tap-weight map (applied on device)."""
    knn = np.asarray(knn_inds)
    taps = np.zeros(4 * K, np.int64)
    wtap = np.zeros((128, 4 * K), np.float32)
    for k in range(knn.shape[0]):
        x = int(knn[k, 0])
        y = int(knn[k, 1])
        for j, (t, s) in enumerate(((0, 0), (0, 1), (1, 0), (1, 1))):
            taps[j * K + k] = (2 * y + t) * W + (2 * x + s)
            wtap[:, j * K + k] = _WVEC[2 * y + t] * _WVEC[2 * x + s]
    return taps, np.ascontiguousarray(wtap.astype(F16_NP))


def _make_in_maps(f1: np.ndarray, f2: np.ndarray, knn_inds: np.ndarray):
    taps, wtap = _tap_tables(knn_inds)
    # rows h=1 and h=54 have exactly-zero bilinear weight (wvec[1] ==
    # wvec[54] == 0): drop them from the shipped tensor (pure slicing)
    f1s = np.asarray(f1, np.float32).astype(F16_NP)[:, :, F1ROWS, :]
    # host-side shard/slice: tap columns of f2 (indexing only, no math);
    # packed partition-major with the weight table so it ships as ONE DMA
    # of 128 large descriptors on the ACT queue, leaving the SP queue a
    # pure f1 stream from t=0
    f2t = np.asarray(f2, np.float32).reshape(B, C, HW)[:, :, taps].astype(F16_NP)
    in_maps = []
    for c in range(NCORES):
        arr = f2t[c * BL : (c + 1) * BL]  # [BL, C, 400]
        m2 = np.ascontiguousarray(
            arr.reshape(BL, NCB, 128, 4 * K)
            .transpose(2, 0, 1, 3)
            .reshape(128, BL * NCB * 4 * K)
        )
        mega = np.ascontiguousarray(np.concatenate([wtap, m2], axis=1))
        in_maps.append(
            {
                "f1": np.ascontiguousarray(f1s[c * BL : (c + 1) * BL]),
                "mega": mega,
                "wf": WF_NP,
            }
        )
    return in_maps


def _build(tc, out_ap, f1_ap, mega_ap, wf_ap):
    nc = tc.nc
    MS = __import__("concourse.bass", fromlist=["MemorySpace"]).MemorySpace

    from contextlib import ExitStack

    with ExitStack() as ctx:
        const = ctx.enter_context(tc.tile_pool(name="const", bufs=1))
        mpool = ctx.enter_context(tc.tile_pool(name="mpool", bufs=2))
        s1p = ctx.enter_context(tc.tile_pool(name="s1p", bufs=2))
        d2selp = ctx.enter_context(tc.tile_pool(name="d2selp", bufs=16))
        tf1p = ctx.enter_context(tc.tile_pool(name="tf1p", bufs=6))
        cbp = ctx.enter_context(tc.tile_pool(name="cbp", bufs=3))
        up = ctx.enter_context(tc.tile_pool(name="up", bufs=3))
        vp = ctx.enter_context(tc.tile_pool(name="vp", bufs=3))
        crawp = ctx.enter_context(tc.tile_pool(name="crawp", bufs=2))
        c28p = ctx.enter_context(tc.tile_pool(name="c28p", bufs=2))
        expbp = ctx.enter_context(tc.tile_pool(name="expbp", bufs=2))
        obp = ctx.enter_context(tc.tile_pool(name="obp", bufs=2))
        smallp = ctx.enter_context(tc.tile_pool(name="smallp", bufs=6))
        cpsp = ctx.enter_context(tc.tile_pool(name="cpsp", bufs=8, space=MS.PSUM))

        NMEGA = 4 * K * (1 + BL * NCB)
        mega = const.tile([128, NMEGA], F16, tag="mega")
        wf = const.tile([128, HS * W], F16, tag="wf")
        nc.scalar.dma_start(mega[:], mega_ap)
        nc.scalar.dma_start(wf[:], wf_ap)
        wtap = mega[:, 0 : 4 * K]

        # ---- f2 taps for ALL batches up front: tiny DMAs + combines, so
        # the PE matmul stream later only ever waits on f1 quarters ----
        d2sel_all = []
        for b in range(BL):
            d2sel_tiles = []
            for i in range(NCB):
                off = 4 * K * (1 + b * NCB + i)
                m = mpool.tile([128, 4 * K], F16, tag="m")
                nc.vector.tensor_mul(m[:], mega[:, off : off + 4 * K], wtap)
                s1 = s1p.tile([128, 2 * K], F16, tag="s1")
                nc.vector.tensor_add(
                    s1[:], m[:, 0 : 2 * K], m[:, 2 * K : 4 * K]
                )
                dsel = d2selp.tile([128, K], F16, tag="d2sel")
                nc.vector.tensor_add(dsel[:], s1[:, 0:K], s1[:, K : 2 * K])
                d2sel_tiles.append(dsel)
            d2sel_all.append(d2sel_tiles)

        for b in range(BL):
            d2sel_tiles = d2sel_all[b]

            # ---- f1 load: four packed quarter-DMAs over 54 shipped rows ----
            tf1q = []
            for q, (r0, nr) in enumerate([(0, 15), (15, 16), (31, 16), (47, 7)]):
                t = tf1p.tile([128, NCB * nr * W], F16, tag=f"tf1q{q}")
                nc.sync.dma_start(
                    t.rearrange("c (g x) -> c g x", g=NCB),
                    f1_ap[b, :, r0 : r0 + nr, :].rearrange(
                        "(g c) h w -> c g (h w)", c=128
                    ),
                )
                tf1q.append(t.rearrange("c (g h w) -> c g h w", g=NCB, h=nr))

            # ---- correlation in 56x56 space + post-matmul downsample ----
            craw = crawp.tile([K, HW28], F32, tag="craw")
            cr3 = craw.rearrange("k (h w) -> k h w", h=S)
            c28 = c28p.tile([K, HW28], F32, tag="c28")
            expb = expbp.tile([K, HW28], BF16, tag="expb")
            den_t = smallp.tile([K, NG], F32, tag="den_t")
            for g in range(NG):  # 7 groups (8 original rows, 7-8 shipped)
                q, lo, nr = GQ[g]
                N = nr * W
                tv = tf1q[q]
                cps = cpsp.tile([K, 8 * W], F32, tag="cps")
                for i in range(NCB):
                    nc.tensor.matmul(
                        cps[:, 0:N],
                        d2sel_tiles[i][:],
                        tv[:, i, lo : lo + nr, :],
                        start=(i == 0),
                        stop=(i == NCB - 1),
                    )
                cb = cbp.tile([K, 8 * W], F16, tag="cb")
                if g % 2 == 0:
                    nc.scalar.copy(cb[:, 0:N], cps[:, 0:N])
                else:
                    nc.vector.tensor_copy(cb[:, 0:N], cps[:, 0:N])
                deng = nc.gpsimd if g in (2, 5) else nc.vector
                u = up.tile([K, 8 * W], F16, tag="u")
                deng.tensor_mul(
                    u[:, 0:N], cb[:, 0:N],
                    wf[0:K, GSTART[g] * W : GSTART[g] * W + N],
                )
                u3 = u.rearrange("k (h w) -> k h w", h=8)
                v = vp.tile([K, 4 * W], F16, tag="v")
                v3 = v.rearrange("k (h w) -> k h w", h=4)
                if g == 0:
                    # orig pair (0,1): row 1 dropped (zero weight)
                    nc.vector.tensor_copy(v3[:, 0:1, :], u3[:, 0:1, :])
                    deng.tensor_add(
                        v3[:, 1:4, :], u3[:, 1:7:2, :], u3[:, 2:7:2, :]
                    )
                elif g == NG - 1:
                    # orig pair (54,55): row 54 dropped (zero weight)
                    deng.tensor_add(
                        v3[:, 0:3, :], u3[:, 0:6:2, :], u3[:, 1:6:2, :]
                    )
                    nc.vector.tensor_copy(v3[:, 3:4, :], u3[:, 6:7, :])
                else:
                    deng.tensor_add(v3, u3[:, 0:8:2, :], u3[:, 1:8:2, :])
                deng.tensor_add(
                    cr3[:, g * 4 : (g + 1) * 4, :],
                    v3[:, :, 0:W:2],
                    v3[:, :, 1:W:2],
                )
                # per-group relu + exp with a partial denominator, so only
                # the reduction/scale chain remains after the last group
                gs = g * 4 * S
                ge = (g + 1) * 4 * S
                nc.scalar.activation(c28[:, gs:ge], craw[:, gs:ge], AF.Relu)
                nc.scalar.activation(
                    expb[:, gs:ge], c28[:, gs:ge], AF.Exp,
                    accum_out=den_t[:, g : g + 1],
                )

            # ---- sum partials, reciprocal, scale by 10/denom ----
            den = smallp.tile([K, 1], F32, tag="den")
            dtmp = smallp.tile([K, NG], F32, tag="dtmp")
            nc.scalar.activation(
                dtmp[:], den_t[:], AF.Identity, accum_out=den[:]
            )
            rec = smallp.tile([K, 1], F32, tag="rec")
            nc.vector.reciprocal(rec[:], den[:])
            rec10 = smallp.tile([K, 1], F32, tag="rec10")
            nc.vector.tensor_scalar_mul(rec10[:], rec[:], 10.0)
            ob = obp.tile([K, HW28], F32, tag="ob")
            nc.scalar.mul(ob[:], c28[:], rec10[:])
            # out DMA on the ACT HWDGE queue (inputs stream on SP)
            nc.scalar.dma_start(out_ap[b], ob[:])


_CACHE: dict = {}


def _get_nc():
    if "nc" in _CACHE:
        return _CACHE["nc"]
    nc = bacc.Bacc(
        "TRN2",
        target_bir_lowering=False,
        debug=False,
        enable_asserts=False,
        num_devices=NCORES,
    )
    f1 = nc.dram_tensor("f1", [BL, C, HS, W], F16, kind="ExternalInput").ap()
    mega = nc.dram_tensor(
        "mega", [128, 4 * K * (1 + BL * NCB)], F16, kind="ExternalInput"
    ).ap()
    wf = nc.dram_tensor("wf", [128, HS * W], F16, kind="ExternalInput").ap()
    out = nc.dram_tensor("out", [BL, K, HW28], F32, kind="ExternalOutput").ap()
    with tile.TileContext(nc) as tc:
        _build(tc, out, f1, mega, wf)
    nc.compile()
    _CACHE["nc"] = nc
    return nc


def kernel(feature1, feature2, knn_inds):
    f1 = np.asarray(feature1, dtype=np.float32)
    f2 = np.asarray(feature2, dtype=np.float32)
    nc = _get_nc()
    in_maps = _make_in_maps(f1, f2, knn_inds)
    res = bass_utils.run_bass_kernel_spmd(nc, in_maps, core_ids=list(range(NCORES)))
    _CACHE["last_results"] = res
    out = np.concatenate([r["out"] for r in res.results], axis=0)
    return out.reshape(B, K, S, S)


# revision 53
# speedup vs baseline: 1.1494x; 1.1494x over previous
"""Trainium2 Bass kernel for nn_CorrelationMapLayer.

reference semantics:
    d1 = bilinear_down28(feature1)            # [B, C, 28, 28]
    d2 = bilinear_down28(feature2)            # [B, C, 28, 28]
    f2_sel[b,c,k] = d2[b, c, y_k, x_k]        # knn gather (y=knn[:,1], x=knn[:,0])
    corr = relu(einsum('bck,bchw->bkhw', f2_sel, d1))
    out  = corr / sum_{h,w} exp(corr) * 10

Kernel structure (final):
  * inputs are cast to f16 on the host; DMA is the roofline.
  * f2 is consumed only through the 4 bilinear taps at the K knn points
    (0.5% of the tensor). The host-side shard step slices those tap
    columns out of f2 (pure indexing -- the bilinear weights and all
    arithmetic stay on device), so each core loads ~13MB instead of ~26MB.
  * All 16 tap-combine stages (one mul + two pair adds per channel block)
    are hoisted ahead of the main loop so the PE matmul stream only ever
    waits on f1 quarter arrivals.
  * f1 feeds the correlation matmul in the original 56x56 space as raw
    f16 (no elementwise work, 4 quarter-DMAs per batch); the bilinear
    downsample is applied AFTER the matmul on corr56 [K=100, 3136]
    (K < C so this is ~5x cheaper than premultiplying f1): psum -> f16
    copy (ACT/DVE alternating), premultiply by the separable weight map,
    h-pair add, strided w-pair add (2 of 7 groups on gpsimd), then
    relu / exp+accumulate / reciprocal / scale-by-10.
  * Queues: input DMAs on the SP HWDGE queue, output + consts on the ACT
    HWDGE queue so neither stream head-of-line blocks the other.
  * Data parallel over batch: 4 batches per core x 8 cores.
"""

import os
import sys

import numpy as np

for _p in (
    "/root/.axon_site",
    "/root/.axon_site/_ro/trn_rl_repo",
    "/root/.axon_site/_ro/pypackages",
    "/opt/trn_rl_repo",
):
    if os.path.isdir(_p) and _p not in sys.path:
        sys.path.append(_p)

import concourse.bacc as bacc
import concourse.mybir as mybir
import concourse.tile as tile
from concourse import bass_utils

F32 = mybir.dt.float32
BF16 = mybir.dt.bfloat16
FP8 = mybir.dt.float8e4
F16 = mybir.dt.float16
F16_NP = mybir.dt.np(mybir.dt.float16)
FP8_NP = mybir.dt.np(mybir.dt.float8e4)
F1SCALE = 1.0
AF = mybir.ActivationFunctionType

B, C, H, W, K = 32, 512, 56, 56, 100
NCORES = 8
BL = B // NCORES  # batches per core
S = 28
HW = H * W  # 3136
HW28 = S * S  # 784
NCB = C // 128  # 4 channel blocks
NG = 7  # corr h-row groups (7 x 8 original rows)
F1ROWS = [0] + list(range(2, 54)) + [55]  # 54 shipped rows (drop 1, 54)
HS = len(F1ROWS)  # 54
# per corr group: (quarter idx, local row start, shipped row count)
GQ = [(0, 0, 7), (0, 7, 8), (1, 0, 8), (1, 8, 8), (2, 0, 8), (2, 8, 8), (3, 0, 7)]
GSTART = [0, 7, 15, 23, 31, 39, 47]  # shipped row offset of each group

BF16_NP = mybir.dt.np(BF16)


def _bilinear_matrix(in_size: int, out_size: int) -> np.ndarray:
    scale = np.float32((in_size - 1) / (out_size - 1)) if out_size > 1 else np.float32(0)
    coords = np.arange(out_size, dtype=np.float32) * scale
    lo = np.floor(coords).astype(np.int32)
    hi = np.minimum(lo + 1, in_size - 1)
    frac = coords - lo.astype(np.float32)
    M = np.zeros((out_size, in_size), np.float32)
    np.add.at(M, (np.arange(out_size), lo), np.float32(1.0) - frac)
    np.add.at(M, (np.arange(out_size), hi), frac)
    return M


def _tap_weights() -> np.ndarray:
    """wvec[w]: weight applied to input index w, whose (unique) consumer is
    output index w//2. Verifies the 2-tap stride-2 structure exactly."""
    M = _bilinear_matrix(H, S)  # [28, 56]
    wvec = np.zeros(H, np.float32)
    for w in range(H):
        wvec[w] = M[w // 2, w]
    M2 = np.zeros_like(M)
    for ow in range(S):
        M2[ow, 2 * ow] = wvec[2 * ow]
        M2[ow, 2 * ow + 1] = wvec[2 * ow + 1]
    assert np.abs(M - M2).max() <= 1e-6, "bilinear 2-tap structure violated"
    return wvec


_WVEC = _tap_weights()
# WF[p, h*56+w] = wvec[h]*wvec[w]  (full separable 2D weight map)
_WF_ROW = (np.repeat(_WVEC, W) * np.tile(_WVEC, H)).astype(np.float32)
_WF54 = _WF_ROW.reshape(H, W)[[0] + list(range(2, 54)) + [55], :].reshape(-1)
WF_NP = np.ascontiguousarray(
    np.broadcast_to(_WF54[None, :], (128, 54 * W)), dtype=F16_NP
)


def _tap_tables(knn_inds: np.ndarray):
    """Flat hw indices of the 4 bilinear taps per knn point (for the host
    slice) + the matching tap-weight map (applied on device)."""
    knn = np.asarray(knn_inds)
    taps = np.zeros(4 * K, np.int64)
    wtap = np.zeros((128, 4 * K), np.float32)
    for k in range(knn.shape[0]):
        x = int(knn[k, 0])
        y = int(knn[k, 1])
        for j, (t, s) in enumerate(((0, 0), (0, 1), (1, 0), (1, 1))):
            taps[j * K + k] = (2 * y + t) * W + (2 * x + s)
            wtap[:, j * K + k] = _WVEC[2 * y + t] * _WVEC[2 * x + s]
    return taps, np.ascontiguousarray(wtap.astype(F16_NP))


def _make_in_maps(f1: np.ndarray, f2: np.ndarray, knn_inds: np.ndarray):
    taps, wtap = _tap_tables(knn_inds)
    # rows h=1 and h=54 have exactly-zero bilinear weight (wvec[1] ==
    # wvec[54] == 0): drop them from the shipped tensor (pure slicing)
    f1s = np.asarray(f1, np.float32).astype(F16_NP)[:, :, F1ROWS, :]
    # host-side shard/slice: tap columns of f2 (indexing only, no math);
    # packed partition-major with the weight table so it ships as ONE DMA
    # of 128 large descriptors on the ACT queue, leaving the SP queue a
    # pure f1 stream from t=0
    f2t = np.asarray(f2, np.float32).reshape(B, C, HW)[:, :, taps].astype(F16_NP)
    in_maps = []
    for c in range(NCORES):
        arr = f2t[c * BL : (c + 1) * BL]  # [BL, C, 400]
        m2 = np.ascontiguousarray(
            arr.reshape(BL, NCB, 128, 4 * K)
            .transpose(2, 0, 1, 3)
            .reshape(128, BL * NCB * 4 * K)
        )
        mega = np.ascontiguousarray(np.concatenate([wtap, m2], axis=1))
        in_maps.append(
            {
                "f1": np.ascontiguousarray(f1s[c * BL : (c + 1) * BL]),
                "mega": mega,
                "wf": WF_NP,
            }
        )
    return in_maps


def _build(tc, out_ap, f1_ap, mega_ap, wf_ap):
    nc = tc.nc
    MS = __import__("concourse.bass", fromlist=["MemorySpace"]).MemorySpace

    from contextlib import ExitStack

    with ExitStack() as ctx:
        const = ctx.enter_context(tc.tile_pool(name="const", bufs=1))
        mpool = ctx.enter_context(tc.tile_pool(name="mpool", bufs=2))
        s1p = ctx.enter_context(tc.tile_pool(name="s1p", bufs=2))
        d2selp = ctx.enter_context(tc.tile_pool(name="d2selp", bufs=16))
        tf1p = ctx.enter_context(tc.tile_pool(name="tf1p", bufs=6))
        cbp = ctx.enter_context(tc.tile_pool(name="cbp", bufs=3))
        up = ctx.enter_context(tc.tile_pool(name="up", bufs=3))
        vp = ctx.enter_context(tc.tile_pool(name="vp", bufs=3))
        crawp = ctx.enter_context(tc.tile_pool(name="crawp", bufs=2))
        c28p = ctx.enter_context(tc.tile_pool(name="c28p", bufs=2))
        expbp = ctx.enter_context(tc.tile_pool(name="expbp", bufs=2))
        obp = ctx.enter_context(tc.tile_pool(name="obp", bufs=2))
        smallp = ctx.enter_context(tc.tile_pool(name="smallp", bufs=6))
        cpsp = ctx.enter_context(tc.tile_pool(name="cpsp", bufs=7, space=MS.PSUM))

        NMEGA = 4 * K * (1 + BL * NCB)
        mega = const.tile([128, NMEGA], F16, tag="mega")
        wf = const.tile([128, HS * W], F16, tag="wf")
        nc.scalar.dma_start(mega[:], mega_ap)
        nc.scalar.dma_start(wf[:], wf_ap)
        wtap = mega[:, 0 : 4 * K]

        # ---- f2 taps for ALL batches up front: tiny DMAs + combines, so
        # the PE matmul stream later only ever waits on f1 quarters ----
        d2sel_all = []
        for b in range(BL):
            d2sel_tiles = []
            for i in range(NCB):
                off = 4 * K * (1 + b * NCB + i)
                m = mpool.tile([128, 4 * K], F16, tag="m")
                nc.vector.tensor_mul(m[:], mega[:, off : off + 4 * K], wtap)
                s1 = s1p.tile([128, 2 * K], F16, tag="s1")
                nc.vector.tensor_add(
                    s1[:], m[:, 0 : 2 * K], m[:, 2 * K : 4 * K]
                )
                dsel = d2selp.tile([128, K], F16, tag="d2sel")
                nc.vector.tensor_add(dsel[:], s1[:, 0:K], s1[:, K : 2 * K])
                d2sel_tiles.append(dsel)
            d2sel_all.append(d2sel_tiles)

        for b in range(BL):
            d2sel_tiles = d2sel_all[b]

            # ---- f1 load: four packed quarter-DMAs over 54 shipped rows ----
            tf1q = []
            for q, (r0, nr) in enumerate([(0, 15), (15, 16), (31, 16), (47, 7)]):
                t = tf1p.tile([128, NCB * nr * W], F16, tag=f"tf1q{q}")
                nc.sync.dma_start(
                    t.rearrange("c (g x) -> c g x", g=NCB),
                    f1_ap[b, :, r0 : r0 + nr, :].rearrange(
                        "(g c) h w -> c g (h w)", c=128
                    ),
                )
                tf1q.append(t.rearrange("c (g h w) -> c g h w", g=NCB, h=nr))

            # ---- correlation in 56x56 space + post-matmul downsample ----
            craw = crawp.tile([K, HW28], F32, tag="craw")
            cr3 = craw.rearrange("k (h w) -> k h w", h=S)
            for g in range(NG):  # 7 groups (8 original rows, 7-8 shipped)
                q, lo, nr = GQ[g]
                N = nr * W
                tv = tf1q[q]
                cps = cpsp.tile([K, 8 * W], F32, tag="cps")
                for i in range(NCB):
                    nc.tensor.matmul(
                        cps[:, 0:N],
                        d2sel_tiles[i][:],
                        tv[:, i, lo : lo + nr, :],
                        start=(i == 0),
                        stop=(i == NCB - 1),
                    )
                cb = cbp.tile([K, 8 * W], F16, tag="cb")
                if g % 2 == 0:
                    nc.scalar.copy(cb[:, 0:N], cps[:, 0:N])
                else:
                    nc.vector.tensor_copy(cb[:, 0:N], cps[:, 0:N])
                deng = nc.gpsimd if g in (2, 5) else nc.vector
                u = up.tile([K, 8 * W], F16, tag="u")
                deng.tensor_mul(
                    u[:, 0:N], cb[:, 0:N],
                    wf[0:K, GSTART[g] * W : GSTART[g] * W + N],
                )
                u3 = u.rearrange("k (h w) -> k h w", h=8)
                v = vp.tile([K, 4 * W], F16, tag="v")
                v3 = v.rearrange("k (h w) -> k h w", h=4)
                if g == 0:
                    # orig pair (0,1): row 1 dropped (zero weight)
                    nc.vector.tensor_copy(v3[:, 0:1, :], u3[:, 0:1, :])
                    deng.tensor_add(
                        v3[:, 1:4, :], u3[:, 1:7:2, :], u3[:, 2:7:2, :]
                    )
                elif g == NG - 1:
                    # orig pair (54,55): row 54 dropped (zero weight)
                    deng.tensor_add(
                        v3[:, 0:3, :], u3[:, 0:6:2, :], u3[:, 1:6:2, :]
                    )
                    nc.vector.tensor_copy(v3[:, 3:4, :], u3[:, 6:7, :])
                else:
                    deng.tensor_add(v3, u3[:, 0:8:2, :], u3[:, 1:8:2, :])
                deng.tensor_add(
                    cr3[:, g * 4 : (g + 1) * 4, :],
                    v3[:, :, 0:W:2],
                    v3[:, :, 1:W:2],
                )

            # ---- relu, exp + accumulate, reciprocal, scale by 10/denom ----
            c28 = c28p.tile([K, HW28], F32, tag="c28")
            nc.scalar.activation(c28[:], craw[:], AF.Relu)
            expb = expbp.tile([K, HW28], BF16, tag="expb")
            den = smallp.tile([K, 1], F32, tag="den")
            nc.scalar.activation(expb[:], c28[:], AF.Exp, accum_out=den[:])
            rec = smallp.tile([K, 1], F32, tag="rec")
            nc.vector.reciprocal(rec[:], den[:])
            rec10 = smallp.tile([K, 1], F32, tag="rec10")
            nc.vector.tensor_scalar_mul(rec10[:], rec[:], 10.0)
            ob = obp.tile([K, HW28], F32, tag="ob")
            nc.scalar.mul(ob[:], c28[:], rec10[:])
            # out DMA on the ACT HWDGE queue (inputs stream on SP)
            nc.scalar.dma_start(out_ap[b], ob[:])


_CACHE: dict = {}


def _get_nc():
    if "nc" in _CACHE:
        return _CACHE["nc"]
    nc = bacc.Bacc(
        "TRN2",
        target_bir_lowering=False,
        debug=False,
        enable_asserts=False,
        num_devices=NCORES,
    )
    f1 = nc.dram_tensor("f1", [BL, C, HS, W], F16, kind="ExternalInput").ap()
    mega = nc.dram_tensor(
        "mega", [128, 4 * K * (1 + BL * NCB)], F16, kind="ExternalInput"
    ).ap()
    wf = nc.dram_tensor("wf", [128, HS * W], F16, kind="ExternalInput").ap()
    out = nc.dram_tensor("out", [BL, K, HW28], F32, kind="ExternalOutput").ap()
    with tile.TileContext(nc) as tc:
        _build(tc, out, f1, mega, wf)
    nc.compile()
    _CACHE["nc"] = nc
    return nc


def kernel(feature1, feature2, knn_inds):
    f1 = np.asarray(feature1, dtype=np.float32)
    f2 = np.asarray(feature2, dtype=np.float32)
    nc = _get_nc()
    in_maps = _make_in_maps(f1, f2, knn_inds)
    res = bass_utils.run_bass_kernel_spmd(nc, in_maps, core_ids=list(range(NCORES)))
    _CACHE["last_results"] = res
    out = np.concatenate([r["out"] for r in res.results], axis=0)
    return out.reshape(B, K, S, S)
